# revision 3
# baseline (speedup 1.0000x reference)
"""Trainium2 Bass kernel v3 for GQA attention (B=2, S=2048, HID=2048, H=16, HKV=4, RoPE, causal).

Sharding: TP=4 over GQA groups x DP=2 over batch (core i -> batch i//4, group i%4);
host sums the 4 partial x@Wo_shard outputs per batch.

v3: software-pipelined strips with TRANSPOSED scores. Per 512-query strip t:
project chunks 4t..4t+3 (QKV matmuls + RoPE + PE transposes into qT/kT/v), then
attention computed as scoresT[sk,sq] = K @ Q^T per key-chunk (kT stationary) ->
transposed-triangle mask -> exp into expT (SBUF fp16, no transposes of P
needed: expT is the PV moving operand directly). scoresT(h) matmuls interleave
per key-chunk with PV + ones-matmul rowsum of head h-1 (variable-width PSUM
accumulation skips the causally-dead regions). Normalization: DVE reciprocal
[1,512] -> PE outer-product broadcast (ones_row x rcp) -> ACT evict to SBUF
(DVE reads only one PSUM operand) -> fused into the attnT eviction multiply.
O-projection of strip t is emitted after proj(t+1) so normalize latency hides
under projection matmuls; output is f16 (host sums partials in f32). PSUM:
ps512 x3 (qkv-accum/scores/rowsum), tp512 x2 (q/k transposes), po512 x2
(pv/o-proj), bc x1 = 8 banks. Inputs DMA per strip, xt interleaved with wqkv.
"""
import sys
sys.path.insert(0, "/opt/trn_rl_repo")
import math
import numpy as np
import concourse.mybir as mybir
import concourse.tile as tile
from concourse import bacc
from concourse.bass_utils import run_bass_kernel_spmd
from concourse.masks import make_identity

F16 = mybir.dt.float16
F32 = mybir.dt.float32
AF = mybir.ActivationFunctionType
ALU = mybir.AluOpType

NH = 4          # q heads per core
D = 128         # head dim
MASK_VAL = -1e9
EXP_BIAS = -4.0

DEFAULT_BUFS = dict(ps512=4, tp512=1, po512=2, bc=1,
                    xt=2, cs=2, rope=2, expT=2, rcp=2, bc16=2, eadd=2, attnT=2, osb=2)


def build(S=2048, HID=2048, repeat=1, bufs=None, interleave=True, do_rsum=True,
          varwidth=True, dmat=False, split_norm=False):  # dmat/split_norm: legacy no-ops
    bz = dict(DEFAULT_BUFS)
    if bufs:
        bz.update(bufs)
    SC = S // 128        # seq chunks (16)
    NT = S // 512        # 512-wide query strips (4)
    HC = HID // 128      # hidden (contraction) chunks (16)
    QW = NH * D          # 512: q width per core
    KVW = 256
    scale = 1.0 / math.sqrt(D)

    nc = bacc.Bacc(None, target_bir_lowering=False, debug=False)
    with tile.TileContext(nc) as tc:
        with tc.tile_pool(name="dram", bufs=1, space="DRAM") as dram:
            xt_d = dram.tile([128, HC * S], F16, kind="ExternalInput", name="xt", uniquify=False)
            wqkv_d = dram.tile([128, HC * (QW + KVW)], F16, kind="ExternalInput", name="wqkv", uniquify=False)
            cos_d = dram.tile([128, SC * QW], F16, kind="ExternalInput", name="cos4", uniquify=False)
            sin_d = dram.tile([128, SC * QW], F16, kind="ExternalInput", name="sin4", uniquify=False)
            wo_d = dram.tile([128, NH * HID], F16, kind="ExternalInput", name="wo", uniquify=False)
            out_d = dram.tile([S, HID], F16, kind="ExternalOutput", name="out", uniquify=False)

            with tc.tile_pool(name="keep", bufs=1) as keep:
                qT_sb = keep.tile([128, NH * S], F16)   # [d, h*S + sq]
                kT_sb = keep.tile([128, S], F16)        # [d, sk]
                v_sb = keep.tile([128, S], F16)         # [sk%128, chunk*128 + d]
                wqkv_sb = keep.tile([128, HC * (QW + KVW)], F16)
                wo_sb = keep.tile([128, NH * HID], F16)
                ident = keep.tile([128, 128], F16)
                make_identity(nc, ident[:])
                ebias = keep.tile([128, 1], F32)
                nc.gpsimd.memset(ebias[:], EXP_BIAS)
                ones_col = keep.tile([128, 1], F16)
                nc.gpsimd.memset(ones_col[:], 1.0)
                ones_row = keep.tile([1, 128], F16)
                nc.gpsimd.memset(ones_row[:], 1.0)
                # transposed triangular causal mask for the diagonal 128x128
                # block of scoresT [sk, sq]: visible (0) iff sq >= sk
                cmaskT = keep.tile([128, 128], F32)
                nc.gpsimd.memset(cmaskT[:], 0.0)
                nc.gpsimd.affine_select(
                    out=cmaskT[:], in_=cmaskT[:], compare_op=ALU.is_ge,
                    fill=MASK_VAL, base=0, pattern=[[1, 128]], channel_multiplier=-1,
                )

                from contextlib import ExitStack
                _rep = ExitStack()
                if repeat > 1:
                    _rep.enter_context(tc.For_i(0, repeat, 1))

                with tc.tile_pool(name="pa", bufs=1) as pa, \
                     tc.tile_pool(name="ps", bufs=1, space="PSUM") as ps:
                    def emit_in_dmas(t):
                        xt_t = pa.tile([128, HC * 512], F16, tag="xt", bufs=bz["xt"])
                        for hh in range(HC):
                            nc.sync.dma_start(
                                out=xt_t[:, hh * 512:(hh + 1) * 512],
                                in_=xt_d[:, hh * S + t * 512: hh * S + (t + 1) * 512])
                            if t == 0:  # first strip: co-prefetch the weight chunk
                                nc.sync.dma_start(
                                    out=wqkv_sb[:, hh * (QW + KVW):(hh + 1) * (QW + KVW)],
                                    in_=wqkv_d[:, hh * (QW + KVW):(hh + 1) * (QW + KVW)])
                        cos_t = pa.tile([128, 4 * QW], F16, tag="cos", bufs=bz["cs"])
                        sin_t = pa.tile([128, 4 * QW], F16, tag="sin", bufs=bz["cs"])
                        nc.sync.dma_start(out=cos_t[:], in_=cos_d[:, 4 * t * QW: 4 * (t + 1) * QW])
                        nc.sync.dma_start(out=sin_t[:], in_=sin_d[:, 4 * t * QW: 4 * (t + 1) * QW])
                        if t == 0:  # wo needed only from the first O-projection on
                            nc.sync.dma_start(out=wo_sb[:], in_=wo_d[:])
                        return xt_t, cos_t, sin_t

                    def proj(t, xt_t, cos_t, sin_t):
                        pend = None
                        if t == 0:
                            # strip 0 is DMA-paced: interleave chunks 0+1 hh-outer so
                            # the PE tracks xt/wqkv chunk arrival instead of waiting
                            # for the full contraction's last chunk per ci.
                            qs = [ps.tile([128, QW], F32, tag="ps512", bufs=bz["ps512"], name=f"q01_{i}")
                                  for i in range(2)]
                            kvs = [ps.tile([128, QW], F32, tag="ps512", bufs=bz["ps512"], name=f"kv01_{i}")
                                   for i in range(2)]
                            for hh in range(HC):
                                for ci in range(2):
                                    xk = xt_t[:, hh * 512 + ci * 128: hh * 512 + (ci + 1) * 128]
                                    nc.tensor.matmul(qs[ci][:], xk,
                                                     wqkv_sb[:, hh * (QW + KVW): hh * (QW + KVW) + QW],
                                                     start=(hh == 0), stop=(hh == HC - 1))
                                for ci in range(2):
                                    xk = xt_t[:, hh * 512 + ci * 128: hh * 512 + (ci + 1) * 128]
                                    nc.tensor.matmul(kvs[ci][:, 0:KVW], xk,
                                                     wqkv_sb[:, hh * (QW + KVW) + QW: (hh + 1) * (QW + KVW)],
                                                     start=(hh == 0), stop=(hh == HC - 1))
                            for ci in range(2):
                                pend = rope_block(ci, qs[ci], kvs[ci], cos_t, sin_t, pend)
                            rest = range(2, 4)
                        else:
                            rest = range(4)
                        for ci in rest:
                            q_ps = ps.tile([128, QW], F32, tag="ps512", bufs=bz["ps512"])
                            kv_ps = ps.tile([128, QW], F32, tag="ps512", bufs=bz["ps512"])
                            for hh in range(HC):
                                xk = xt_t[:, hh * 512 + ci * 128: hh * 512 + (ci + 1) * 128]
                                nc.tensor.matmul(q_ps[:], xk,
                                                 wqkv_sb[:, hh * (QW + KVW): hh * (QW + KVW) + QW],
                                                 start=(hh == 0), stop=(hh == HC - 1))
                            for hh in range(HC):
                                xk = xt_t[:, hh * 512 + ci * 128: hh * 512 + (ci + 1) * 128]
                                nc.tensor.matmul(kv_ps[:, 0:KVW], xk,
                                                 wqkv_sb[:, hh * (QW + KVW) + QW: (hh + 1) * (QW + KVW)],
                                                 start=(hh == 0), stop=(hh == HC - 1))
                            pend = rope_block(4 * t + ci, q_ps, kv_ps, cos_t, sin_t, pend)
                        emit_transposes(*pend)

                    def rope_block(c, q_ps, kv_ps, cos_t, sin_t, pend):
                        """RoPE q/k + persist v; emits the PREVIOUS chunk's transposes
                        (delayed one chunk so PE never stalls on the RoPE DVE chain)."""
                        ci = c % 4
                        # --- RoPE on q (4 heads batched) ---
                        q4 = q_ps[:].rearrange("p (h d) -> p h d", h=NH)
                        sin4v = sin_t[:, ci * QW:(ci + 1) * QW].rearrange("p (h d) -> p h d", h=NH)
                        rot = pa.tile([128, QW], F32, tag="rot", bufs=bz["rope"])
                        rot4 = rot[:].rearrange("p (h d) -> p h d", h=NH)
                        nc.vector.tensor_mul(rot4[:, :, 0:64], q4[:, :, 64:128], sin4v[:, :, 0:64])
                        nc.vector.tensor_mul(rot4[:, :, 64:128], q4[:, :, 0:64], sin4v[:, :, 64:128])
                        qc = pa.tile([128, QW], F32, tag="qc", bufs=bz["rope"])
                        nc.vector.tensor_mul(qc[:], q_ps[:], cos_t[:, ci * QW:(ci + 1) * QW])
                        q16 = pa.tile([128, QW], F16, tag="q16", bufs=bz["rope"])
                        nc.vector.tensor_add(q16[:], qc[:], rot[:])
                        # --- RoPE on k (head 0 slices of cos/sin) ---
                        k1 = kv_ps[:, 0:128]
                        cos1 = cos_t[:, ci * QW: ci * QW + 128]
                        sin1 = sin_t[:, ci * QW: ci * QW + 128]
                        krot = pa.tile([128, 128], F32, tag="krot", bufs=bz["rope"])
                        nc.vector.tensor_mul(krot[:, 0:64], k1[:, 64:128], sin1[:, 0:64])
                        nc.vector.tensor_mul(krot[:, 64:128], k1[:, 0:64], sin1[:, 64:128])
                        kc = pa.tile([128, 128], F32, tag="kc", bufs=bz["rope"])
                        nc.vector.tensor_mul(kc[:], k1, cos1)
                        k16 = pa.tile([128, 128], F16, tag="k16", bufs=bz["rope"])
                        nc.vector.tensor_add(k16[:], kc[:], krot[:])
                        # --- v to persistent [s, d] fp16 ---
                        nc.vector.tensor_copy(v_sb[:, c * 128:(c + 1) * 128], kv_ps[:, 128:KVW])
                        if pend is not None:
                            emit_transposes(*pend)
                        return (c, q16, k16)

                    def emit_transposes(c, q16, k16):
                        """q heads + k chunk -> qT/kT via PE transpose (tp512 tag)."""
                        tq = ps.tile([128, 512], F16, tag="tp512", bufs=bz["tp512"])
                        for h in range(NH):
                            nc.tensor.transpose(tq[:, h * 128:(h + 1) * 128],
                                                q16[:, h * 128:(h + 1) * 128], ident[:])
                        qT_view = qT_sb[:].rearrange("p (h s) -> p h s", h=NH)[:, :, c * 128:(c + 1) * 128]
                        nc.vector.tensor_copy(qT_view, tq[:].rearrange("p (h s) -> p h s", h=NH))
                        tk = ps.tile([128, 512], F16, tag="tp512", bufs=bz["tp512"])
                        nc.tensor.transpose(tk[:, 0:128], k16[:], ident[:])
                        nc.vector.tensor_copy(kT_sb[:, c * 128:(c + 1) * 128], tk[:, 0:128])

                    def attn_stage(t, h, prev_exp):
                        """Interleaved per key-chunk: scoresT(h) matmul -> mask -> exp,
                        with PV + rowsum matmuls of head h-1 riding between, so the PE
                        never outruns ACT's exp into the ps512 rotation.

                        scoresT[sk,sq] = K @ Q^T (kT chunk stationary); exp writes expT
                        in SBUF fp16 which is the PV/rowsum moving operand directly.
                        Diagonal chunks compute only from their visible query offset."""
                        nk = 4 * t + 4
                        if h is not None:
                            expT_h = pa.tile([128, nk * 512], F16, tag="expT", bufs=bz["expT"])
                            qTs = qT_sb[:, h * S + t * 512: h * S + (t + 1) * 512]
                        else:
                            expT_h = None
                            qTs = None
                        if prev_exp is not None:
                            pv = ps.tile([128, 512], F32, tag="po512", bufs=bz["po512"])
                            rsum = ps.tile([128, 512], F32, tag="ps512", bufs=bz["ps512"])

                        def emit_scores(k):
                            off = max(0, k - 4 * t) * 128
                            w = 512 - off
                            sc = ps.tile([128, 512], F32, tag="ps512", bufs=bz["ps512"])
                            nc.tensor.matmul(sc[:, 0:w], kT_sb[:, k * 128:(k + 1) * 128],
                                             qTs[:, off:512], start=True, stop=True)
                            if k >= 4 * t:
                                nc.vector.tensor_add(sc[:, 0:128], sc[:, 0:128], cmaskT[:])
                            nc.scalar.activation(expT_h[:, k * 512 + off:(k + 1) * 512],
                                                 sc[:, 0:w], AF.Exp, scale=scale, bias=ebias[:])

                        rsum_started = [False]

                        def emit_rsum(rhs, off, stop):
                            nc.tensor.matmul(rsum[0:1, off:512], ones_col[:], rhs,
                                             start=not rsum_started[0], stop=stop)
                            rsum_started[0] = True

                        def emit_pv(k):
                            off = (max(0, k - 4 * t) * 128) if varwidth else 0
                            nc.tensor.matmul(pv[:, off:512], v_sb[:, k * 128:(k + 1) * 128],
                                             prev_exp[:, k * 512 + off:(k + 1) * 512],
                                             start=(k == 0), stop=(k == nk - 1))
                            if not do_rsum:
                                return
                            if k < 4 * t:
                                # full chunks: pre-add pairs on DVE (elementwise over the
                                # partition-aligned key index; the ones-matmul sums over
                                # partitions, so pair-summing first is exact) -> half the
                                # rowsum matmul columns on PE.
                                if k % 2 == 1:
                                    eadd = pa.tile([128, 512], F16, tag="eadd", bufs=bz["eadd"])
                                    nc.vector.tensor_add(eadd[:], prev_exp[:, (k - 1) * 512:k * 512],
                                                         prev_exp[:, k * 512:(k + 1) * 512])
                                    emit_rsum(eadd[:], 0, stop=False)
                            else:
                                off = max(0, k - 4 * t) * 128
                                emit_rsum(prev_exp[:, k * 512 + off:(k + 1) * 512], off,
                                          stop=(k == nk - 1))

                        for k in range(nk):
                            if h is not None:
                                emit_scores(k)
                            if prev_exp is not None and interleave:
                                emit_pv(k)
                        if prev_exp is not None and not interleave:
                            for k in range(nk):
                                emit_pv(k)
                        if prev_exp is not None:
                            rcp = pa.tile([1, 512], F16, tag="rcp", bufs=bz["rcp"])
                            with nc.allow_low_precision("softmax reciprocal to f16 for PE broadcast"):
                                if do_rsum:
                                    nc.vector.reciprocal(rcp[:], rsum[0:1, :])
                                else:
                                    nc.gpsimd.memset(rcp[:], 1.0)
                            return expT_h, (pv, rcp)
                        return expT_h, None

                    def norm_stage(h, pv, rcp, attnT):
                        """outer-product broadcast of 1/rowsum on PE, normalize on eviction.
                        (bc must round-trip to SBUF: DVE reads only one PSUM operand.)"""
                        bc = ps.tile([128, 512], F32, tag="bc", bufs=bz["bc"])
                        nc.tensor.matmul(bc[:], ones_row[:], rcp[:], start=True, stop=True)
                        bc16 = pa.tile([128, 512], F16, tag="bc16", bufs=bz["bc16"])
                        nc.scalar.copy(bc16[:], bc[:])
                        nc.vector.tensor_mul(attnT[:, h * 512:(h + 1) * 512], pv[:], bc16[:])

                    def oproj(t, attnT):
                        for ci in range(4):
                            c = 4 * t + ci
                            osb = pa.tile([128, HID], F16, tag="osb", bufs=bz["osb"])
                            for n in range(HID // 512):
                                op = ps.tile([128, 512], F32, tag="po512", bufs=bz["po512"])
                                for h in range(NH):
                                    nc.tensor.matmul(op[:], attnT[:, h * 512 + ci * 128: h * 512 + (ci + 1) * 128],
                                                     wo_sb[:, h * HID + n * 512: h * HID + (n + 1) * 512],
                                                     start=(h == 0), stop=(h == NH - 1))
                                # alternate eviction engine to balance ACT/DVE load
                                (nc.scalar.copy if n % 2 == 0 else nc.vector.tensor_copy)(
                                    osb[:, n * 512:(n + 1) * 512], op[:])
                            nc.sync.dma_start(out=out_d[c * 128:(c + 1) * 128, :], in_=osb[:])

                    cur = emit_in_dmas(0)
                    pending_oproj = None
                    for t in range(NT):
                        nxt = emit_in_dmas(t + 1) if t + 1 < NT else None
                        proj(t, *cur)
                        if pending_oproj is not None:
                            oproj(t - 1, pending_oproj)
                        attnT = pa.tile([128, NH * 512], F16, tag="attnT", bufs=bz["attnT"])
                        prev = None
                        pending_norm = None
                        for h in range(NH):
                            expT_h, done = attn_stage(t, h, prev)
                            if pending_norm is not None:
                                norm_stage(*pending_norm, attnT)
                            if done is not None:
                                pending_norm = (h - 1, done[0], done[1])
                            prev = expT_h
                        _, done = attn_stage(t, None, prev)
                        norm_stage(*pending_norm, attnT)
                        norm_stage(NH - 1, done[0], done[1], attnT)
                        pending_oproj = attnT
                        cur = nxt
                    oproj(NT - 1, pending_oproj)
                _rep.close()
    nc.compile()
    return nc


def _chunk_major(a, rows=128):
    """[R, C] -> [128, (R//128)*C] with row-chunk-major free layout."""
    r, c = a.shape
    return np.ascontiguousarray(a.reshape(r // rows, rows, c).transpose(1, 0, 2).reshape(rows, (r // rows) * c))


def make_in_map(x_b, cos, sin, wq_g, wk_g, wv_g, wo_g, S, HID):
    SC = S // 128
    xt = _chunk_major(np.ascontiguousarray(x_b.T)).astype(np.float16)
    wqkv = _chunk_major(np.concatenate([wq_g, wk_g, wv_g], axis=1)).astype(np.float16)
    cosr = cos[:S].reshape(SC, 128, D)
    cos4 = np.repeat(cosr[:, :, None, :], NH, axis=2).transpose(1, 0, 2, 3).reshape(128, SC * NH * D)
    sing = np.concatenate([-sin[:S, :64], sin[:S, 64:]], axis=1).reshape(SC, 128, D)
    sin4 = np.repeat(sing[:, :, None, :], NH, axis=2).transpose(1, 0, 2, 3).reshape(128, SC * NH * D)
    wo = _chunk_major(wo_g).astype(np.float16)
    return {
        "xt": xt,
        "wqkv": wqkv,
        "cos4": np.ascontiguousarray(cos4).astype(np.float16),
        "sin4": np.ascontiguousarray(sin4).astype(np.float16),
        "wo": wo,
    }


_NC_CACHE = {}

BEST_BUFS = {}
BEST_DMAT = False
BEST_SPLIT_NORM = False


def _get_nc(S, HID):
    key = (S, HID)
    if key not in _NC_CACHE:
        _NC_CACHE[key] = build(S, HID, bufs=BEST_BUFS)
    return _NC_CACHE[key]


def kernel(x, cos, sin, Wq, Wk, Wv, Wo):
    x = np.asarray(x, dtype=np.float32)
    cos = np.asarray(cos, dtype=np.float32)
    sin = np.asarray(sin, dtype=np.float32)
    Wq = np.asarray(Wq, dtype=np.float32)
    Wk = np.asarray(Wk, dtype=np.float32)
    Wv = np.asarray(Wv, dtype=np.float32)
    Wo = np.asarray(Wo, dtype=np.float32)
    B, S, HID = x.shape

    in_maps = []
    for i in range(8):
        b, g = i // 4, i % 4
        in_maps.append(make_in_map(
            x[b], cos, sin,
            Wq[:, g * NH * D:(g + 1) * NH * D],
            Wk[:, g * D:(g + 1) * D],
            Wv[:, g * D:(g + 1) * D],
            Wo[g * NH * D:(g + 1) * NH * D, :],
            S, HID))

    nc = _get_nc(S, HID)
    last_err = None
    for _attempt in range(3):
        try:
            res = run_bass_kernel_spmd(nc, in_maps, core_ids=list(range(8)), trace=False)
            break
        except Exception as e:  # flaky NRT_EXEC_UNIT_UNRECOVERABLE seen on first runs
            last_err = e
            import time as _time
            _time.sleep(5.0)
    else:
        raise last_err
    out = np.zeros((B, S, HID), dtype=np.float32)
    for i in range(8):
        b = i // 4
        out[b] += res.results[i]["out"].astype(np.float32)
    return out


# revision 5
# speedup vs baseline: 2.0358x; 2.0358x over previous
"""Trainium2 Bass kernel v3 for GQA attention (B=2, S=2048, HID=2048, H=16, HKV=4, RoPE, causal).

Sharding: TP=4 over GQA groups x DP=2 over batch (core i -> batch i//4, group i%4);
host sums the 4 partial x@Wo_shard outputs per batch.

v3: software-pipelined strips with TRANSPOSED scores. Per 512-query strip t:
project chunks 4t..4t+3 (QKV matmuls + RoPE + PE transposes into qT/kT/v), then
attention computed as scoresT[sk,sq] = K @ Q^T per key-chunk (kT stationary) ->
transposed-triangle mask -> exp into expT (SBUF fp16, no transposes of P
needed: expT is the PV moving operand directly). scoresT(h) matmuls interleave
per key-chunk with PV + ones-matmul rowsum of head h-1 (variable-width PSUM
accumulation skips the causally-dead regions; full key-chunk pairs are
pre-summed on DVE so the rowsum matmul columns halve). Normalization: DVE reciprocal
[1,512] -> PE outer-product broadcast (ones_row x rcp) -> ACT evict to SBUF
(DVE reads only one PSUM operand) -> fused into the attnT eviction multiply.
O-projection of strip t is emitted after proj(t+1) so normalize latency hides
under projection matmuls; output is f16 (host sums partials in f32). PSUM:
ps512 x4 (qkv-accum/scores/rowsum), tp512 x1 (q/k transposes), po512 x2
(pv/o-proj), bc x1 = 8 banks. Inputs DMA per strip, xt interleaved with wqkv.
"""
import sys
sys.path.insert(0, "/opt/trn_rl_repo")
import math
import numpy as np
import concourse.mybir as mybir
import concourse.tile as tile
from concourse import bacc
from concourse.bass_utils import run_bass_kernel_spmd
from concourse.masks import make_identity

F16 = mybir.dt.float16
F32 = mybir.dt.float32
AF = mybir.ActivationFunctionType
ALU = mybir.AluOpType

NH = 4          # q heads per core
D = 128         # head dim
MASK_VAL = -1e9
EXP_BIAS = -4.0

DEFAULT_BUFS = dict(ps512=4, tp512=1, po512=2, bc=1,
                    xt=2, cs=2, rope=2, expT=2, rcp=2, bc16=2, eadd=2, attnT=2, osb=2)


def build(S=2048, HID=2048, repeat=1, bufs=None, interleave=True, do_rsum=True,
          varwidth=True, dmat=False, split_norm=False):  # dmat/split_norm: legacy no-ops
    bz = dict(DEFAULT_BUFS)
    if bufs:
        bz.update(bufs)
    SC = S // 128        # seq chunks (16)
    NT = S // 512        # 512-wide query strips (4)
    HC = HID // 128      # hidden (contraction) chunks (16)
    QW = NH * D          # 512: q width per core
    KVW = 256
    scale = 1.0 / math.sqrt(D)

    nc = bacc.Bacc(None, target_bir_lowering=False, debug=False)
    with tile.TileContext(nc) as tc:
        with tc.tile_pool(name="dram", bufs=1, space="DRAM") as dram:
            xt_d = dram.tile([128, HC * S], F16, kind="ExternalInput", name="xt", uniquify=False)
            wqkv_d = dram.tile([128, HC * (QW + KVW)], F16, kind="ExternalInput", name="wqkv", uniquify=False)
            cos_d = dram.tile([128, SC * QW], F16, kind="ExternalInput", name="cos4", uniquify=False)
            sin_d = dram.tile([128, SC * QW], F16, kind="ExternalInput", name="sin4", uniquify=False)
            wo_d = dram.tile([128, NH * HID], F16, kind="ExternalInput", name="wo", uniquify=False)
            out_d = dram.tile([S, HID], F16, kind="ExternalOutput", name="out", uniquify=False)

            with tc.tile_pool(name="keep", bufs=1) as keep:
                qT_sb = keep.tile([128, NH * S], F16)   # [d, h*S + sq]
                kT_sb = keep.tile([128, S], F16)        # [d, sk]
                v_sb = keep.tile([128, S], F16)         # [sk%128, chunk*128 + d]
                wqkv_sb = keep.tile([128, HC * (QW + KVW)], F16)
                wo_sb = keep.tile([128, NH * HID], F16)
                ident = keep.tile([128, 128], F16)
                make_identity(nc, ident[:])
                ebias = keep.tile([128, 1], F32)
                nc.gpsimd.memset(ebias[:], EXP_BIAS)
                ones_col = keep.tile([128, 1], F16)
                nc.gpsimd.memset(ones_col[:], 1.0)
                ones_row = keep.tile([1, 128], F16)
                nc.gpsimd.memset(ones_row[:], 1.0)
                # transposed triangular causal mask for the diagonal 128x128
                # block of scoresT [sk, sq]: visible (0) iff sq >= sk
                cmaskT = keep.tile([128, 128], F32)
                nc.gpsimd.memset(cmaskT[:], 0.0)
                nc.gpsimd.affine_select(
                    out=cmaskT[:], in_=cmaskT[:], compare_op=ALU.is_ge,
                    fill=MASK_VAL, base=0, pattern=[[1, 128]], channel_multiplier=-1,
                )

                from contextlib import ExitStack
                _rep = ExitStack()
                if repeat > 1:
                    _rep.enter_context(tc.For_i(0, repeat, 1))

                with tc.tile_pool(name="pa", bufs=1) as pa, \
                     tc.tile_pool(name="ps", bufs=1, space="PSUM") as ps:
                    def emit_in_dmas(t):
                        xt_t = pa.tile([128, HC * 512], F16, tag="xt", bufs=bz["xt"])
                        for hh in range(HC):
                            nc.sync.dma_start(
                                out=xt_t[:, hh * 512:(hh + 1) * 512],
                                in_=xt_d[:, hh * S + t * 512: hh * S + (t + 1) * 512])
                            if t == 0:  # first strip: co-prefetch the weight chunk
                                nc.sync.dma_start(
                                    out=wqkv_sb[:, hh * (QW + KVW):(hh + 1) * (QW + KVW)],
                                    in_=wqkv_d[:, hh * (QW + KVW):(hh + 1) * (QW + KVW)])
                        cos_t = pa.tile([128, 4 * QW], F16, tag="cos", bufs=bz["cs"])
                        sin_t = pa.tile([128, 4 * QW], F16, tag="sin", bufs=bz["cs"])
                        nc.sync.dma_start(out=cos_t[:], in_=cos_d[:, 4 * t * QW: 4 * (t + 1) * QW])
                        nc.sync.dma_start(out=sin_t[:], in_=sin_d[:, 4 * t * QW: 4 * (t + 1) * QW])
                        if t == 0:  # wo needed only from the first O-projection on
                            nc.sync.dma_start(out=wo_sb[:], in_=wo_d[:])
                        return xt_t, cos_t, sin_t

                    def proj(t, xt_t, cos_t, sin_t):
                        pend = None
                        if t == 0:
                            # strip 0 is DMA-paced: interleave chunks 0+1 hh-outer so
                            # the PE tracks xt/wqkv chunk arrival instead of waiting
                            # for the full contraction's last chunk per ci.
                            qs = [ps.tile([128, QW], F32, tag="ps512", bufs=bz["ps512"], name=f"q01_{i}")
                                  for i in range(2)]
                            kvs = [ps.tile([128, QW], F32, tag="ps512", bufs=bz["ps512"], name=f"kv01_{i}")
                                   for i in range(2)]
                            for hh in range(HC):
                                for ci in range(2):
                                    xk = xt_t[:, hh * 512 + ci * 128: hh * 512 + (ci + 1) * 128]
                                    nc.tensor.matmul(qs[ci][:], xk,
                                                     wqkv_sb[:, hh * (QW + KVW): hh * (QW + KVW) + QW],
                                                     start=(hh == 0), stop=(hh == HC - 1))
                                for ci in range(2):
                                    xk = xt_t[:, hh * 512 + ci * 128: hh * 512 + (ci + 1) * 128]
                                    nc.tensor.matmul(kvs[ci][:, 0:KVW], xk,
                                                     wqkv_sb[:, hh * (QW + KVW) + QW: (hh + 1) * (QW + KVW)],
                                                     start=(hh == 0), stop=(hh == HC - 1))
                            for ci in range(2):
                                pend = rope_block(ci, qs[ci], kvs[ci], cos_t, sin_t, pend)
                            rest = range(2, 4)
                        else:
                            rest = range(4)
                        for ci in rest:
                            q_ps = ps.tile([128, QW], F32, tag="ps512", bufs=bz["ps512"])
                            kv_ps = ps.tile([128, QW], F32, tag="ps512", bufs=bz["ps512"])
                            for hh in range(HC):
                                xk = xt_t[:, hh * 512 + ci * 128: hh * 512 + (ci + 1) * 128]
                                nc.tensor.matmul(q_ps[:], xk,
                                                 wqkv_sb[:, hh * (QW + KVW): hh * (QW + KVW) + QW],
                                                 start=(hh == 0), stop=(hh == HC - 1))
                            for hh in range(HC):
                                xk = xt_t[:, hh * 512 + ci * 128: hh * 512 + (ci + 1) * 128]
                                nc.tensor.matmul(kv_ps[:, 0:KVW], xk,
                                                 wqkv_sb[:, hh * (QW + KVW) + QW: (hh + 1) * (QW + KVW)],
                                                 start=(hh == 0), stop=(hh == HC - 1))
                            pend = rope_block(4 * t + ci, q_ps, kv_ps, cos_t, sin_t, pend)
                        emit_transposes(*pend)

                    def rope_block(c, q_ps, kv_ps, cos_t, sin_t, pend):
                        """Evict q/kv PSUM to fp16 SBUF on ACT (frees the banks fast),
                        then RoPE in fp16 at DVE 2x; emits the PREVIOUS chunk's
                        transposes (delayed so PE never stalls on the RoPE chain)."""
                        ci = c % 4
                        q16c = pa.tile([128, QW], F16, tag="q16c", bufs=bz["rope"])
                        nc.scalar.copy(q16c[:], q_ps[:])
                        kv16c = pa.tile([128, KVW], F16, tag="kv16c", bufs=bz["rope"])
                        nc.scalar.copy(kv16c[:], kv_ps[:, 0:KVW])
                        # --- RoPE on q (4 heads batched, fp16 2x) ---
                        q4 = q16c[:].rearrange("p (h d) -> p h d", h=NH)
                        sin4v = sin_t[:, ci * QW:(ci + 1) * QW].rearrange("p (h d) -> p h d", h=NH)
                        rot = pa.tile([128, QW], F16, tag="rot", bufs=bz["rope"])
                        rot4 = rot[:].rearrange("p (h d) -> p h d", h=NH)
                        nc.vector.tensor_mul(rot4[:, :, 0:64], q4[:, :, 64:128], sin4v[:, :, 0:64])
                        nc.vector.tensor_mul(rot4[:, :, 64:128], q4[:, :, 0:64], sin4v[:, :, 64:128])
                        qc = pa.tile([128, QW], F16, tag="qc", bufs=bz["rope"])
                        nc.vector.tensor_mul(qc[:], q16c[:], cos_t[:, ci * QW:(ci + 1) * QW])
                        q16 = pa.tile([128, QW], F16, tag="q16", bufs=bz["rope"])
                        nc.vector.tensor_add(q16[:], qc[:], rot[:])
                        # --- RoPE on k (head 0 slices of cos/sin, fp16 2x) ---
                        k1 = kv16c[:, 0:128]
                        cos1 = cos_t[:, ci * QW: ci * QW + 128]
                        sin1 = sin_t[:, ci * QW: ci * QW + 128]
                        krot = pa.tile([128, 128], F16, tag="krot", bufs=bz["rope"])
                        nc.vector.tensor_mul(krot[:, 0:64], k1[:, 64:128], sin1[:, 0:64])
                        nc.vector.tensor_mul(krot[:, 64:128], k1[:, 0:64], sin1[:, 64:128])
                        kc = pa.tile([128, 128], F16, tag="kc", bufs=bz["rope"])
                        nc.vector.tensor_mul(kc[:], k1, cos1)
                        k16 = pa.tile([128, 128], F16, tag="k16", bufs=bz["rope"])
                        nc.vector.tensor_add(k16[:], kc[:], krot[:])
                        # --- v to persistent [s, d] fp16 (from the SBUF eviction, so the
                        # kv PSUM bank frees right after the ACT copy) ---
                        nc.vector.tensor_copy(v_sb[:, c * 128:(c + 1) * 128], kv16c[:, 128:KVW])
                        if pend is not None:
                            emit_transposes(*pend)
                        return (c, q16, k16)

                    def emit_transposes(c, q16, k16):
                        """q heads + k chunk -> qT/kT via PE transpose (tp512 tag)."""
                        tq = ps.tile([128, 512], F16, tag="tp512", bufs=bz["tp512"])
                        for h in range(NH):
                            nc.tensor.transpose(tq[:, h * 128:(h + 1) * 128],
                                                q16[:, h * 128:(h + 1) * 128], ident[:])
                        qT_view = qT_sb[:].rearrange("p (h s) -> p h s", h=NH)[:, :, c * 128:(c + 1) * 128]
                        nc.vector.tensor_copy(qT_view, tq[:].rearrange("p (h s) -> p h s", h=NH))
                        tk = ps.tile([128, 512], F16, tag="tp512", bufs=bz["tp512"])
                        nc.tensor.transpose(tk[:, 0:128], k16[:], ident[:])
                        nc.vector.tensor_copy(kT_sb[:, c * 128:(c + 1) * 128], tk[:, 0:128])

                    def attn_stage(t, h, prev_exp):
                        """Interleaved per key-chunk: scoresT(h) matmul -> mask -> exp,
                        with PV + rowsum matmuls of head h-1 riding between, so the PE
                        never outruns ACT's exp into the ps512 rotation.

                        scoresT[sk,sq] = K @ Q^T (kT chunk stationary); exp writes expT
                        in SBUF fp16 which is the PV/rowsum moving operand directly.
                        Diagonal chunks compute only from their visible query offset."""
                        nk = 4 * t + 4
                        if h is not None:
                            expT_h = pa.tile([128, nk * 512], F16, tag="expT", bufs=bz["expT"])
                            qTs = qT_sb[:, h * S + t * 512: h * S + (t + 1) * 512]
                        else:
                            expT_h = None
                            qTs = None
                        if prev_exp is not None:
                            pv = ps.tile([128, 512], F32, tag="po512", bufs=bz["po512"])
                            rsum = ps.tile([128, 512], F32, tag="ps512", bufs=bz["ps512"])

                        def emit_scores(k):
                            off = max(0, k - 4 * t) * 128
                            w = 512 - off
                            sc = ps.tile([128, 512], F32, tag="ps512", bufs=bz["ps512"])
                            nc.tensor.matmul(sc[:, 0:w], kT_sb[:, k * 128:(k + 1) * 128],
                                             qTs[:, off:512], start=True, stop=True)
                            if k >= 4 * t:
                                nc.vector.tensor_add(sc[:, 0:128], sc[:, 0:128], cmaskT[:])
                            nc.scalar.activation(expT_h[:, k * 512 + off:(k + 1) * 512],
                                                 sc[:, 0:w], AF.Exp, scale=scale, bias=ebias[:])

                        rsum_started = [False]

                        def emit_rsum(rhs, off, stop):
                            nc.tensor.matmul(rsum[0:1, off:512], ones_col[:], rhs,
                                             start=not rsum_started[0], stop=stop)
                            rsum_started[0] = True

                        def emit_pv(k):
                            off = (max(0, k - 4 * t) * 128) if varwidth else 0
                            nc.tensor.matmul(pv[:, off:512], v_sb[:, k * 128:(k + 1) * 128],
                                             prev_exp[:, k * 512 + off:(k + 1) * 512],
                                             start=(k == 0), stop=(k == nk - 1))
                            if not do_rsum:
                                return
                            if k < 4 * t:
                                # full chunks: pre-add pairs on DVE (elementwise over the
                                # partition-aligned key index; the ones-matmul sums over
                                # partitions, so pair-summing first is exact) -> half the
                                # rowsum matmul columns on PE.
                                if k % 2 == 1:
                                    eadd = pa.tile([128, 512], F16, tag="eadd", bufs=bz["eadd"])
                                    nc.vector.tensor_add(eadd[:], prev_exp[:, (k - 1) * 512:k * 512],
                                                         prev_exp[:, k * 512:(k + 1) * 512])
                                    emit_rsum(eadd[:], 0, stop=False)
                            else:
                                off = max(0, k - 4 * t) * 128
                                emit_rsum(prev_exp[:, k * 512 + off:(k + 1) * 512], off,
                                          stop=(k == nk - 1))

                        for k in range(nk):
                            if h is not None:
                                emit_scores(k)
                            if prev_exp is not None and interleave:
                                emit_pv(k)
                        if prev_exp is not None and not interleave:
                            for k in range(nk):
                                emit_pv(k)
                        if prev_exp is not None:
                            rcp = pa.tile([1, 512], F16, tag="rcp", bufs=bz["rcp"])
                            with nc.allow_low_precision("softmax reciprocal to f16 for PE broadcast"):
                                if do_rsum:
                                    nc.vector.reciprocal(rcp[:], rsum[0:1, :])
                                else:
                                    nc.gpsimd.memset(rcp[:], 1.0)
                            return expT_h, (pv, rcp)
                        return expT_h, None

                    def norm_stage(h, pv, rcp, attnT):
                        """outer-product broadcast of 1/rowsum on PE, normalize on eviction.
                        (bc must round-trip to SBUF: DVE reads only one PSUM operand.)"""
                        bc = ps.tile([128, 512], F32, tag="bc", bufs=bz["bc"])
                        nc.tensor.matmul(bc[:], ones_row[:], rcp[:], start=True, stop=True)
                        bc16 = pa.tile([128, 512], F16, tag="bc16", bufs=bz["bc16"])
                        nc.scalar.copy(bc16[:], bc[:])
                        nc.vector.tensor_mul(attnT[:, h * 512:(h + 1) * 512], pv[:], bc16[:])

                    def oproj(t, attnT):
                        for ci in range(4):
                            c = 4 * t + ci
                            osb = pa.tile([128, HID], F16, tag="osb", bufs=bz["osb"])
                            for n in range(HID // 512):
                                op = ps.tile([128, 512], F32, tag="po512", bufs=bz["po512"])
                                for h in range(NH):
                                    nc.tensor.matmul(op[:], attnT[:, h * 512 + ci * 128: h * 512 + (ci + 1) * 128],
                                                     wo_sb[:, h * HID + n * 512: h * HID + (n + 1) * 512],
                                                     start=(h == 0), stop=(h == NH - 1))
                                # alternate eviction engine to balance ACT/DVE load
                                (nc.scalar.copy if n % 2 == 0 else nc.vector.tensor_copy)(
                                    osb[:, n * 512:(n + 1) * 512], op[:])
                            nc.sync.dma_start(out=out_d[c * 128:(c + 1) * 128, :], in_=osb[:])

                    cur = emit_in_dmas(0)
                    pending_oproj = None
                    for t in range(NT):
                        nxt = emit_in_dmas(t + 1) if t + 1 < NT else None
                        proj(t, *cur)
                        if pending_oproj is not None:
                            oproj(t - 1, pending_oproj)
                        attnT = pa.tile([128, NH * 512], F16, tag="attnT", bufs=bz["attnT"])
                        prev = None
                        pending_norm = None
                        for h in range(NH):
                            expT_h, done = attn_stage(t, h, prev)
                            if pending_norm is not None:
                                norm_stage(*pending_norm, attnT)
                            if done is not None:
                                pending_norm = (h - 1, done[0], done[1])
                            prev = expT_h
                        _, done = attn_stage(t, None, prev)
                        norm_stage(*pending_norm, attnT)
                        norm_stage(NH - 1, done[0], done[1], attnT)
                        pending_oproj = attnT
                        cur = nxt
                    oproj(NT - 1, pending_oproj)
                _rep.close()
    nc.compile()
    return nc


def _chunk_major(a, rows=128):
    """[R, C] -> [128, (R//128)*C] with row-chunk-major free layout."""
    r, c = a.shape
    return np.ascontiguousarray(a.reshape(r // rows, rows, c).transpose(1, 0, 2).reshape(rows, (r // rows) * c))


def make_in_map(x_b, cos, sin, wq_g, wk_g, wv_g, wo_g, S, HID):
    SC = S // 128
    xt = _chunk_major(np.ascontiguousarray(x_b.T)).astype(np.float16)
    wqkv = _chunk_major(np.concatenate([wq_g, wk_g, wv_g], axis=1)).astype(np.float16)
    cosr = cos[:S].reshape(SC, 128, D)
    cos4 = np.repeat(cosr[:, :, None, :], NH, axis=2).transpose(1, 0, 2, 3).reshape(128, SC * NH * D)
    sing = np.concatenate([-sin[:S, :64], sin[:S, 64:]], axis=1).reshape(SC, 128, D)
    sin4 = np.repeat(sing[:, :, None, :], NH, axis=2).transpose(1, 0, 2, 3).reshape(128, SC * NH * D)
    wo = _chunk_major(wo_g).astype(np.float16)
    return {
        "xt": xt,
        "wqkv": wqkv,
        "cos4": np.ascontiguousarray(cos4).astype(np.float16),
        "sin4": np.ascontiguousarray(sin4).astype(np.float16),
        "wo": wo,
    }


_NC_CACHE = {}

BEST_BUFS = {}
BEST_DMAT = False
BEST_SPLIT_NORM = False


def _get_nc(S, HID):
    key = (S, HID)
    if key not in _NC_CACHE:
        _NC_CACHE[key] = build(S, HID, bufs=BEST_BUFS)
    return _NC_CACHE[key]


def kernel(x, cos, sin, Wq, Wk, Wv, Wo):
    x = np.asarray(x, dtype=np.float32)
    cos = np.asarray(cos, dtype=np.float32)
    sin = np.asarray(sin, dtype=np.float32)
    Wq = np.asarray(Wq, dtype=np.float32)
    Wk = np.asarray(Wk, dtype=np.float32)
    Wv = np.asarray(Wv, dtype=np.float32)
    Wo = np.asarray(Wo, dtype=np.float32)
    B, S, HID = x.shape

    in_maps = []
    for i in range(8):
        b, g = i // 4, i % 4
        in_maps.append(make_in_map(
            x[b], cos, sin,
            Wq[:, g * NH * D:(g + 1) * NH * D],
            Wk[:, g * D:(g + 1) * D],
            Wv[:, g * D:(g + 1) * D],
            Wo[g * NH * D:(g + 1) * NH * D, :],
            S, HID))

    nc = _get_nc(S, HID)
    last_err = None
    for _attempt in range(3):
        try:
            res = run_bass_kernel_spmd(nc, in_maps, core_ids=list(range(8)), trace=False)
            break
        except Exception as e:  # flaky NRT_EXEC_UNIT_UNRECOVERABLE seen on first runs
            last_err = e
            import time as _time
            _time.sleep(5.0)
    else:
        raise last_err
    out = np.zeros((B, S, HID), dtype=np.float32)
    for i in range(8):
        b = i // 4
        out[b] += res.results[i]["out"].astype(np.float32)
    return out
